# revision 13
# baseline (speedup 1.0000x reference)
"""Single-head causal attention (B=8, T=2048, C=1024, HS=64) on 8 TRN2 cores.

Sharding: one batch element per NeuronCore (pure data parallel, no
collectives).  Inside each core:

  xT [C, T] resident in SBUF (host supplies x[b].T so the contraction dim
  is on partitions).
  qkT [128, T]: rows 0:64 = q^T, rows 64:128 = k^T, computed with a single
  packed projection (lhsT = [w_q | w_k]) in fp32r at full PE rate.
  v  [T, 64] bf16, computed as v^T then PE-transposed.
  Per 128-query block qb (causal => keys [0, (qb+1)*128)):
    S = q^T.T @ k^T in PSUM (fp32r, 512-wide chunks)
    mask+rowmax fused on DVE (tensor_tensor_reduce)
    P = exp(8*S - 8*rowmax) on ScalarE (bf16 out, fused row-sum accum_out)
    P^T per 128-key block via PE transpose -> bf16 SBUF
    out_psum += P^T.T @ v  (bf16 matmuls, fp32 PSUM accumulate)
    out = out_psum * (1/rowsum)  -> DMA to DRAM
"""

import numpy as np

import concourse.bass as bass
import concourse.mybir as mybir
import concourse.tile as tile
from concourse import bacc
from concourse.bass_utils import run_bass_kernel_spmd

B, T, C, HS = 8, 2048, 1024, 64
N_CORES = 8
QB = 128           # query block rows
NQB = T // QB      # 16
NCB = C // 128     # 8 contraction blocks
F32 = mybir.dt.float32
F32R = mybir.dt.float32r
BF16 = mybir.dt.bfloat16
F16 = mybir.dt.float16
NEG_BIG = -1.0e30

PROJ_F32 = False  # run projections as plain fp32 matmuls (4x slower, precise)
S_F32 = False     # run score matmuls as plain fp32 matmuls

AF = mybir.ActivationFunctionType
ALU = mybir.AluOpType


def build_nc() -> bass.Bass:
    nc = bacc.Bacc(
        "TRN2",
        target_bir_lowering=False,
        debug=False,
        num_devices=N_CORES,
    )
    xT = nc.dram_tensor("xT", [C, T], F32R, kind="ExternalInput").ap()
    wqk = nc.dram_tensor("wqk", [C, 2 * HS], F32R, kind="ExternalInput").ap()
    wvq = nc.dram_tensor("wvq", [C, 2 * HS], F32R, kind="ExternalInput").ap()
    dmask = nc.dram_tensor("dmask", [QB, QB], F32, kind="ExternalInput").ap()
    ident = nc.dram_tensor("ident", [128, 128], F32, kind="ExternalInput").ap()
    out = nc.dram_tensor("out", [T, HS], F32, kind="ExternalOutput").ap()

    with tile.TileContext(nc) as tc:
        _body(tc, xT, wqk, wvq, dmask, ident, out)
    nc.compile()
    return nc


def _body(tc: tile.TileContext, xT, wqk, wvq, dmask, ident, out):
    nc = tc.nc
    with (
        tc.tile_pool(name="const", bufs=1) as constp,
        tc.tile_pool(name="xt", bufs=1) as xtp,
        tc.tile_pool(name="w", bufs=1) as wp,
        tc.tile_pool(name="qkv", bufs=1) as qkvp,
        tc.tile_pool(name="loop", bufs=2) as loopp,
        tc.tile_pool(name="stat", bufs=3) as statp,
    ):
        # ---- constants (host-provided: affine_select is broken on HW) ----
        id_f32 = constp.tile([128, 128], F32)
        nc.sync.dma_start(id_f32[:, :], ident[:, :])
        id_f16 = constp.tile([128, 128], F16)
        nc.vector.tensor_copy(id_f16[:, :], id_f32[:, :])
        id_f32r = constp.tile([64, 64], F32R)
        nc.vector.tensor_copy(id_f32r[:, :], id_f32[:64, :64])
        # diagonal-block causal mask: dmask[p, c] = 0 if c <= p else -1e30
        mask = constp.tile([QB, QB], F32)
        nc.sync.dma_start(mask[:, :], dmask[:, :])

        # ---- load inputs ----
        xt = xtp.tile([128, NCB, T], F32R)  # [p, cb, t] = x[t, cb*128+p]
        xT_g = xT.rearrange("(a p) t -> p a t", p=128)
        for cb in range(NCB):
            nc.sync.dma_start(xt[:, cb, :], xT_g[:, cb, :])
        wqk_t = wp.tile([128, NCB, 2 * HS], F32R)
        nc.sync.dma_start(wqk_t[:, :, :], wqk.rearrange("(a p) m -> p a m", p=128))
        wvq_t = wp.tile([128, NCB, 2 * HS], F32R)
        nc.sync.dma_start(wvq_t[:, :, :], wvq.rearrange("(a p) m -> p a m", p=128))

        # ---- projections ----
        # vqk[0:64, 0:T] = v^T; vqk[64:128, 0:T] = q^T; vqk[64:128, T:2T] = k^T
        # (q^T and k^T share base partition 64 as required by matmul)
        vqk = qkvp.tile([128, 2 * T], F32R)
        v_t = qkvp.tile([128, NQB, HS], F16)  # v natural, fp16
        with tc.tile_pool(name="psproj", bufs=2, space="PSUM") as psproj:
            for j in range(T // 512):
                ps = psproj.tile([128, 512], F32, tag="qk")
                sl = slice(512 * j, 512 * (j + 1))
                for cb in range(NCB):
                    lh, rh = wvq_t[:, cb, :], xt[:, cb, sl]
                    if PROJ_F32:
                        lh, rh = lh.bitcast(F32), rh.bitcast(F32)
                    nc.tensor.matmul(
                        ps[:, :], lh, rh,
                        start=(cb == 0),
                        stop=(cb == NCB - 1),
                    )
                nc.scalar.copy(vqk[0:64, sl], ps[0:64, :])
                nc.scalar.copy(vqk[64:128, sl], ps[64:128, :])
            for j in range(T // 512):
                ps = psproj.tile([128, 512], F32, tag="qk")
                sl = slice(512 * j, 512 * (j + 1))
                for cb in range(NCB):
                    lh, rh = wqk_t[:, cb, :], xt[:, cb, sl]
                    if PROJ_F32:
                        lh, rh = lh.bitcast(F32), rh.bitcast(F32)
                    nc.tensor.matmul(
                        ps[:, :], lh, rh,
                        start=(cb == 0),
                        stop=(cb == NCB - 1),
                    )
                nc.scalar.copy(vqk[64:128, T + 512 * j : T + 512 * (j + 1)], ps[64:128, :])
            # v^T -> v (natural layout, bf16) via PE transpose
            for tb in range(NQB):
                pst = psproj.tile([128, HS], F32R, tag="vt")
                nc.tensor.transpose(
                    pst[:, :],
                    vqk[0:64, QB * tb : QB * (tb + 1)],
                    id_f32r[:, :],
                )
                nc.vector.tensor_copy(v_t[:, tb, :], pst[:, :])

        # ---- attention ----
        with (
            tc.tile_pool(name="pss", bufs=1, space="PSUM") as pss,
            tc.tile_pool(name="pst", bufs=2, space="PSUM") as pstp,
            tc.tile_pool(name="pso", bufs=2, space="PSUM") as psop,
        ):
            for qb in range(NQB):
                w = QB * (qb + 1)  # number of keys visible to this block
                s_ps = pss.tile([128, T], F32, tag="s")
                for n0 in range(0, w, 512):
                    n1 = min(w, n0 + 512)
                    lh = vqk[64:128, QB * qb : QB * (qb + 1)]
                    rh = vqk[64:128, T + n0 : T + n1]
                    if S_F32:
                        lh, rh = lh.bitcast(F32), rh.bitcast(F32)
                    nc.tensor.matmul(
                        s_ps[:, n0:n1], lh, rh,
                        start=True,
                        stop=True,
                    )
                # causal mask on the diagonal block, then row-max
                nc.vector.tensor_add(
                    s_ps[:, w - QB : w], s_ps[:, w - QB : w], mask[:, :]
                )
                mrow = statp.tile([128, 1], F32, tag="m")
                nc.vector.reduce_max(
                    mrow[:, :], s_ps[:, :w], axis=mybir.AxisListType.X
                )
                nbias = statp.tile([128, 1], F32, tag="nb")
                nc.vector.tensor_scalar_mul(nbias[:, :], mrow[:, :], -8.0)
                # P = exp(8*s - 8*m), bf16; lrow = row sums
                lrow = statp.tile([128, 1], F32, tag="l")
                p_t = loopp.tile([128, T], F16, tag="p")
                nc.scalar.activation(
                    p_t[:, :w],
                    s_ps[:, :w],
                    AF.Exp,
                    bias=nbias[:, :],
                    scale=8.0,
                    accum_out=lrow[:, :],
                )
                rrow = statp.tile([128, 1], F32, tag="r")
                nc.vector.reciprocal(rrow[:, :], lrow[:, :])
                # transpose P blocks and accumulate P^T.T @ v
                pt_t = loopp.tile([128, T], F16, tag="ptr")
                o_ps = psop.tile([128, HS], F32, tag="o")
                for kb in range(qb + 1):
                    sl = slice(QB * kb, QB * (kb + 1))
                    tp = pstp.tile([128, 128], F16, tag="tp")
                    nc.tensor.transpose(tp[:, :], p_t[:, sl], id_f16[:, :])
                    nc.scalar.copy(pt_t[:, sl], tp[:, :])
                for kb in range(qb + 1):
                    sl = slice(QB * kb, QB * (kb + 1))
                    nc.tensor.matmul(
                        o_ps[:, :],
                        pt_t[:, sl],
                        v_t[:, kb, :],
                        start=(kb == 0),
                        stop=(kb == qb),
                    )
                o_sb = statp.tile([128, HS], F32, tag="osb")
                nc.vector.tensor_scalar_mul(o_sb[:, :], o_ps[:, :], rrow[:, :])
                nc.sync.dma_start(out[QB * qb : QB * (qb + 1), :], o_sb[:, :])


_NC_CACHE = None


def _get_nc():
    global _NC_CACHE
    if _NC_CACHE is None:
        _NC_CACHE = build_nc()
    return _NC_CACHE


def _make_in_maps(input_tensor, w_q, w_k, w_v):
    x = np.asarray(input_tensor, dtype=np.float32)
    wq = np.asarray(w_q, dtype=np.float32)
    wk = np.asarray(w_k, dtype=np.float32)
    wv = np.asarray(w_v, dtype=np.float32)
    wqk = np.ascontiguousarray(np.concatenate([wq, wk], axis=1))
    wvq = np.ascontiguousarray(np.concatenate([wv, wq], axis=1))
    dmask = np.where(
        np.arange(QB)[:, None] >= np.arange(QB)[None, :], 0.0, NEG_BIG
    ).astype(np.float32)
    ident = np.eye(128, dtype=np.float32)
    return [
        {
            "xT": np.ascontiguousarray(x[b].T),
            "wqk": wqk,
            "wvq": wvq,
            "dmask": dmask,
            "ident": ident,
        }
        for b in range(B)
    ]


def run(in_maps, trace=False, tmpdir=None):
    nc = _get_nc()
    return run_bass_kernel_spmd(
        nc, in_maps, list(range(N_CORES)), trace=trace, tmpdir=tmpdir
    )


def kernel(input_tensor, w_q, w_k, w_v):
    in_maps = _make_in_maps(input_tensor, w_q, w_k, w_v)
    res = run(in_maps)
    return np.stack([res.results[b]["out"] for b in range(B)], axis=0)
